# revision 1
# baseline (speedup 1.0000x reference)
"""Trainium2 Bass kernel for nn_LstmModel: B=512, T=256, H=512 LSTM + 2-layer FC head.

Strategy (DP-8): shard batch across 8 cores (64 rows each), replicate weights.
Everything SBUF-resident. Per step t:
  gates[64, 2048] (PSUM, 4 banks; gate bank order i,g,f,o) accumulated on PE:
    - K=2 matmul folds x_t*W_ih + bias  (stationary [x_t; ones], moving [W_ih.T; bias])
    - 4 K-chunks of h @ W_hh.T          (stationary hT chunk, moving W_hh.T chunk)
    All matmuls float32r (1 cycle/row for N>=256).
  ACT: sigmoid/tanh per bank; DVE: c = F*c + I*G, h = O*tanh(c).
  PE transposes h -> hT for next step's stationary.
"""

import sys
from contextlib import ExitStack

if "/opt/trn_rl_repo" not in sys.path:
    sys.path.insert(0, "/opt/trn_rl_repo")

import numpy as np

import concourse.bass as bass
import concourse.tile as tile
from concourse import bacc, mybir
from concourse.bass_utils import run_bass_kernel_spmd
from concourse.masks import make_identity

F32 = mybir.dt.float32
F32R = mybir.dt.float32r
AF = mybir.ActivationFunctionType
ALU = mybir.AluOpType

B, T, H, HALF, TGT = 512, 256, 512, 256, 28
NCORES = 8
BL = B // NCORES          # 64 batch rows per core
G4 = 4 * H                # 2048
NBANK = 4                 # gate banks of 512
KCH = H // 128            # 4 K-chunks

# bank order i, g, f, o (PyTorch row order is i, f, g, o)
_PERM = np.concatenate([
    np.arange(0, H),              # i
    np.arange(2 * H, 3 * H),      # g
    np.arange(H, 2 * H),          # f
    np.arange(3 * H, 4 * H),      # o
])
BANK_I, BANK_G, BANK_F, BANK_O = 0, 1, 2, 3

_cached = {}


def _r(ap):
    return ap.bitcast(F32R)


def build_program():
    nc = bacc.Bacc("TRN2", target_bir_lowering=False, debug=False,
                   num_devices=NCORES)

    d_seq = nc.dram_tensor("seqones", [2, T * BL], F32R, kind="ExternalInput")
    d_whh = nc.dram_tensor("whhT", [128, KCH * G4], F32R, kind="ExternalInput")
    d_wxb = nc.dram_tensor("wxb", [2, G4], F32R, kind="ExternalInput")
    d_f1w = nc.dram_tensor("fc1wT", [128, KCH * HALF], F32R, kind="ExternalInput")
    d_f1b = nc.dram_tensor("fc1b", [1, HALF], F32R, kind="ExternalInput")
    d_f2w = nc.dram_tensor("fc2wT", [128, 2 * TGT], F32R, kind="ExternalInput")
    d_f2b = nc.dram_tensor("fc2b", [1, TGT], F32R, kind="ExternalInput")
    d_out = nc.dram_tensor("out", [BL, TGT], F32, kind="ExternalOutput")

    with tile.TileContext(nc) as tc, ExitStack() as ctx:
        consts = ctx.enter_context(tc.tile_pool(name="consts", bufs=1))
        acts = ctx.enter_context(tc.tile_pool(name="acts", bufs=2))
        state = ctx.enter_context(tc.tile_pool(name="state", bufs=2))
        pg = ctx.enter_context(tc.tile_pool(name="pg", bufs=8, space="PSUM"))

        # ---- load constants into SBUF (dtype f32r; bits are plain fp32)
        seqp = ctx.enter_context(tc.tile_pool(name="seqp", bufs=2))
        sb_whh = consts.tile([128, KCH * G4], F32R)
        sb_wxb = consts.tile([2, G4], F32R)
        sb_f1w = consts.tile([128, KCH * HALF], F32R)
        sb_f1b = consts.tile([1, HALF], F32R)
        sb_f2w = consts.tile([128, 2 * TGT], F32R)
        sb_f2b = consts.tile([1, TGT], F32R)
        ident = consts.tile([128, 128], F32)
        sb_ones_f = consts.tile([1, BL], F32)
        sb_ones = consts.tile([1, BL], F32R)
        nc.gpsimd.memset(sb_ones_f[:], 1.0)
        nc.vector.tensor_copy(sb_ones[:], sb_ones_f[:])
        for dst, dsrc in ((sb_whh, d_whh), (sb_wxb, d_wxb),
                          (sb_f1w, d_f1w), (sb_f1b, d_f1b), (sb_f2w, d_f2w),
                          (sb_f2b, d_f2b)):
            nc.sync.dma_start(dst[:], dsrc.ap())
        make_identity(nc, ident[:])

        c_prev = None
        TCH = 64
        sb_seq = None
        gb = None          # PSUM gate banks for current step (emitted in prev iter)
        HB = 256           # tail half size

        def emit_xmm(t):
            nonlocal sb_seq
            if t % TCH == 0:
                sb_seq = seqp.tile([2, TCH * BL], F32R, tag="sq")
                nc.sync.dma_start(sb_seq[:], d_seq.ap()[:, t * BL:(t + TCH) * BL])
            tt = t % TCH
            xs = sb_seq[:][:, tt * BL:(tt + 1) * BL]
            banks = []
            for nb in range(NBANK):
                g = pg.tile([BL, 512], F32, tag="g")
                nc.tensor.matmul(g[:], xs, sb_wxb[:][:, nb * 512:(nb + 1) * 512],
                                 start=True, stop=(t == 0))
                banks.append(g)
            return banks

        gb = emit_xmm(0)

        for t in range(T):
            if t > 0:
                # h-part matmuls (bank-major; accumulate onto x+bias)
                for nb in range(NBANK):
                    for k in range(KCH):
                        nc.tensor.matmul(
                            gb[nb][:],
                            (hT_a if k < 2 else hT_b)[:][:, (k % 2) * BL:(k % 2 + 1) * BL],
                            sb_whh[:][:, k * G4 + nb * 512: k * G4 + (nb + 1) * 512],
                            start=False, stop=(k == KCH - 1))

            I = acts.tile([BL, 512], F32, tag="I")
            G = acts.tile([BL, 512], F32, tag="G")
            Fg = acts.tile([BL, 512], F32, tag="F")
            O = acts.tile([BL, 512], F32, tag="O")
            Tc = acts.tile([BL, 512], F32, tag="Tc")
            t1 = acts.tile([BL, 512], F32, tag="t1")
            t2 = acts.tile([BL, 512], F32, tag="t2")
            c = state.tile([BL, 512], F32, tag="c")
            h = state.tile([BL, 512], F32, tag="h")

            # ACT queue: i, g, f1, f2, o, tc1, tc2, (copy1)
            nc.scalar.activation(I[:], gb[BANK_I][:], AF.Sigmoid)
            nc.scalar.activation(G[:], gb[BANK_G][:], AF.Tanh)
            nc.gpsimd.tensor_mul(t1[:], I[:], G[:])       # Pool (idle), early
            if t > 0:
                nc.scalar.activation(Fg[:][:, 0:HB], gb[BANK_F][:][:, 0:HB], AF.Sigmoid)
                nc.scalar.activation(Fg[:][:, HB:512], gb[BANK_F][:][:, HB:512], AF.Sigmoid)
                nc.vector.tensor_mul(t2[:][:, 0:HB], Fg[:][:, 0:HB], c_prev[:][:, 0:HB])
                nc.vector.tensor_mul(t2[:][:, HB:512], Fg[:][:, HB:512], c_prev[:][:, HB:512])
            if t > 0:
                nc.vector.tensor_add(c[:][:, 0:HB], t1[:][:, 0:HB], t2[:][:, 0:HB])
                nc.vector.tensor_add(c[:][:, HB:512], t1[:][:, HB:512], t2[:][:, HB:512])
            else:
                nc.vector.tensor_copy(c[:][:, 0:HB], t1[:][:, 0:HB])
                nc.vector.tensor_copy(c[:][:, HB:512], t1[:][:, HB:512])
            nc.scalar.activation(O[:], gb[BANK_O][:], AF.Sigmoid)
            nc.scalar.activation(Tc[:][:, 0:HB], c[:][:, 0:HB], AF.Tanh)
            nc.scalar.activation(Tc[:][:, HB:512], c[:][:, HB:512], AF.Tanh)
            nc.vector.tensor_mul(h[:][:, 0:HB], O[:][:, 0:HB], Tc[:][:, 0:HB])
            nc.vector.tensor_mul(h[:][:, HB:512], O[:][:, HB:512], Tc[:][:, HB:512])

            # prefetch next step's x-part into fresh PSUM banks (fills PE
            # during this step's tail)
            if t + 1 < T:
                gb = emit_xmm(t + 1)

            # transpose h -> hT (two independent halves for early MM re-entry)
            pT_a = pg.tile([128, 2 * BL], F32, tag="g")
            pT_b = pg.tile([128, 2 * BL], F32, tag="g")
            for k in range(2):
                nc.tensor.transpose(pT_a[:, k * BL:(k + 1) * BL],
                                    h[:][:, k * 128:(k + 1) * 128],
                                    ident[:][0:BL, 0:BL])
            for k in range(2):
                nc.tensor.transpose(pT_b[:, k * BL:(k + 1) * BL],
                                    h[:][:, 256 + k * 128:256 + (k + 1) * 128],
                                    ident[:][0:BL, 0:BL])
            hT_a = state.tile([128, 2 * BL], F32R, tag="hTa")
            hT_b = state.tile([128, 2 * BL], F32R, tag="hTb")
            nc.vector.tensor_copy(hT_a[:], pT_a[:])
            nc.vector.tensor_copy(hT_b[:], pT_b[:])

            c_prev = c

        # ---- FC head: hid = relu(h @ fc1_w.T + b1); out = hid @ fc2_w.T + b2
        ones = sb_ones[:]
        p_hid = pg.tile([BL, HALF], F32, tag="g")
        nc.tensor.matmul(p_hid[:], ones, sb_f1b[:], start=True, stop=False)
        for k in range(KCH):
            nc.tensor.matmul(p_hid[:],
                             (hT_a if k < 2 else hT_b)[:][:, (k % 2) * BL:(k % 2 + 1) * BL],
                             sb_f1w[:][:, k * HALF:(k + 1) * HALF],
                             start=False, stop=(k == KCH - 1))
        hid = acts.tile([BL, HALF], F32, tag="hid")
        nc.scalar.activation(hid[:], p_hid[:], AF.Relu)

        pTh = pg.tile([128, 2 * BL], F32, tag="g")
        for k in range(2):
            nc.tensor.transpose(pTh[:, k * BL:(k + 1) * BL],
                                hid[:][:, k * 128:(k + 1) * 128],
                                ident[:][0:BL, 0:BL])
        hidT = acts.tile([128, 2 * BL], F32R, tag="hidT")
        nc.vector.tensor_copy(hidT[:], pTh[:])

        p_out = pg.tile([BL, TGT], F32, tag="g")
        nc.tensor.matmul(p_out[:], ones, sb_f2b[:], start=True, stop=False)
        for k in range(2):
            nc.tensor.matmul(p_out[:], hidT[:][:, k * BL:(k + 1) * BL],
                             sb_f2w[:][:, k * TGT:(k + 1) * TGT],
                             start=False, stop=(k == 1))
        res = acts.tile([BL, TGT], F32, tag="res")
        nc.vector.tensor_copy(res[:], p_out[:])
        nc.sync.dma_start(d_out.ap(), res[:])

    if not nc.is_finalized():
        nc.finalize()
    return nc


def _prep_shared(W_ih, W_hh, b_ih, b_hh, fc1_w, fc1_b, fc2_w, fc2_b):
    Wp = W_hh[_PERM, :].astype(np.float32)                      # [2048, 512]
    whhT = np.empty((128, KCH * G4), np.float32)
    for k in range(KCH):
        whhT[:, k * G4:(k + 1) * G4] = Wp[:, k * 128:(k + 1) * 128].T
    wxb = np.stack([W_ih[_PERM, 0], (b_ih + b_hh)[_PERM]]).astype(np.float32)
    f1w = np.empty((128, KCH * HALF), np.float32)
    for k in range(KCH):
        f1w[:, k * HALF:(k + 1) * HALF] = fc1_w[:, k * 128:(k + 1) * 128].T
    f2w = np.empty((128, 2 * TGT), np.float32)
    for k in range(2):
        f2w[:, k * TGT:(k + 1) * TGT] = fc2_w[:, k * 128:(k + 1) * 128].T
    return {
        "whhT": whhT, "wxb": wxb,
        "fc1wT": f1w, "fc1b": fc1_b.reshape(1, HALF).astype(np.float32),
        "fc2wT": f2w, "fc2b": fc2_b.reshape(1, TGT).astype(np.float32),
    }


def run(inputs, trace=False):
    if "nc" not in _cached:
        _cached["nc"] = build_program()
    nc = _cached["nc"]
    shared = _prep_shared(
        inputs["W_ih"], inputs["W_hh"], inputs["b_ih"], inputs["b_hh"],
        inputs["fc1_w"], inputs["fc1_b"], inputs["fc2_w"], inputs["fc2_b"])
    seq = np.asarray(inputs["sequence"], np.float32)[:, :, 0]   # [B, T]
    in_maps = []
    for cid in range(NCORES):
        xs = seq[cid * BL:(cid + 1) * BL, :].T.reshape(1, T * BL)  # t-major
        seqones = np.concatenate([xs, np.ones_like(xs)], axis=0)
        in_maps.append({"seqones": seqones, **shared})
    br = run_bass_kernel_spmd(nc, in_maps, list(range(NCORES)), trace=trace)
    out = np.concatenate([br.results[i]["out"] for i in range(NCORES)], axis=0)
    return out[:, :, None].astype(np.float32), br


def kernel(**inputs):
    out, _ = run(inputs)
    return out



# revision 18
# speedup vs baseline: 1.3818x; 1.3818x over previous
"""Trainium2 Bass kernel for nn_LstmModel: B=512, T=256, H=512 LSTM + 2-layer FC head.

Strategy (DP-8): shard batch across 8 cores (BL=64 rows each), replicate weights.
Everything SBUF-resident, all matmuls bf16 (1 cyc/col vs 2 for fp32r on HW).

Key layout: each gate lives in a PSUM "quarter" tile [128, 256]:
  rows 0:64   = batch x gate-cols 0:256   (half A)
  rows 64:128 = batch x gate-cols 256:512 (half B)
Two gates share one 2KB PSUM bank: bank_IF = [I | F], bank_GO = [G | O].

Matmuls are emitted as col-tiled concurrent PAIRS (tile_position (0,0) and
(0,64)): the stationary hT chunk is duplicated into both 64-col halves of the
PE array (the transpose uses a [I64|I64] identity so the duplicate is free),
and the two MMs stream different W column-halves into partition rows 0:64 /
64:128 simultaneously -> full 128x128 array utilization at M=64.

Gate bias is folded into the x-part matmul with K=3 stationary [x; 1; 1] and
moving rows [w_ih; b_hi; b_lo] (hi/lo bf16 split keeps bias fp32-accurate).

Elementwise chain runs in [128, 128] halves (L = h-dims {0:128, 256:384},
R = {128:256, 384:512}) so the c -> tanh -> h -> transpose -> cast tail for L
lands early and the next step's K0/K2 matmuls start while R still drains.
"""

import sys
from contextlib import ExitStack

if "/opt/trn_rl_repo" not in sys.path:
    sys.path.insert(0, "/opt/trn_rl_repo")

import numpy as np
import ml_dtypes

import concourse.bass as bass
import concourse.tile as tile
from concourse import bacc, mybir
from concourse.bass_utils import run_bass_kernel_spmd

F32 = mybir.dt.float32
BF16 = mybir.dt.bfloat16
AF = mybir.ActivationFunctionType
BFNP = ml_dtypes.bfloat16

B, T, H, HALF, TGT = 512, 256, 512, 256, 28
NCORES = 8
BL = B // NCORES          # 64 batch rows per core
TCH = 64                  # seq steps per DMA chunk

# gate order in emission / weight layout; torch row offsets in W_hh (i,f,g,o)
GATES = ("i", "g", "f", "o")
GROW = {"i": 0, "f": 512, "g": 1024, "o": 1536}
# gate -> (bank, colhalf): bank_IF holds i (cols 0:256) + f (256:512); GO: g, o
GBANK = {"i": (0, 0), "f": (0, 1), "g": (1, 0), "o": (1, 1)}
KORDER = (0, 2, 1, 3)     # L-chunks first (hT_L ready before hT_R)

_cached = {}
DBG = None        # set to a step index to dump that step's h/c tiles


def build_program():
    nc = bacc.Bacc("TRN2", target_bir_lowering=False, debug=False,
                   num_devices=NCORES)

    d_sx = nc.dram_tensor("sx", [6, T * 128], BF16, kind="ExternalInput")
    d_whh = nc.dram_tensor("whh", [128, 8192], BF16, kind="ExternalInput")
    d_wx = nc.dram_tensor("wx", [6, 1024], BF16, kind="ExternalInput")
    d_iden = nc.dram_tensor("iden", [128, 128], BF16, kind="ExternalInput")
    d_f1w = nc.dram_tensor("f1w", [128, 1024], BF16, kind="ExternalInput")
    d_f1b = nc.dram_tensor("f1b", [2, HALF], BF16, kind="ExternalInput")
    d_f2w = nc.dram_tensor("f2w", [128, 2 * TGT], BF16, kind="ExternalInput")
    d_f2b = nc.dram_tensor("f2b", [2, TGT], BF16, kind="ExternalInput")
    d_out = nc.dram_tensor("out", [BL, TGT], F32, kind="ExternalOutput")
    d_dbg_h = d_dbg_c = d_dbg_g = None
    if DBG is not None:
        d_dbg_h = nc.dram_tensor("dbg_h", [128, 256], F32, kind="ExternalOutput")
        d_dbg_c = nc.dram_tensor("dbg_c", [128, 256], F32, kind="ExternalOutput")
        d_dbg_g = nc.dram_tensor("dbg_g", [128, 1024], F32, kind="ExternalOutput")

    with tile.TileContext(nc) as tc, ExitStack() as ctx:
        consts = ctx.enter_context(tc.tile_pool(name="consts", bufs=1))
        seqp = ctx.enter_context(tc.tile_pool(name="seqp", bufs=2))
        acts = ctx.enter_context(tc.tile_pool(name="acts", bufs=2))
        state = ctx.enter_context(tc.tile_pool(name="state", bufs=2))
        pg = ctx.enter_context(tc.tile_pool(name="pg", bufs=2, space="PSUM"))

        sb_whh = consts.tile([128, 8192], BF16)
        sb_wx = consts.tile([6, 1024], BF16)
        sb_iden = consts.tile([128, 128], BF16)
        sb_f1w = consts.tile([128, 1024], BF16)
        sb_f1b = consts.tile([2, HALF], BF16)
        sb_f2w = consts.tile([128, 2 * TGT], BF16)
        sb_f2b = consts.tile([2, TGT], BF16)
        for dst, dsrc in ((sb_whh, d_whh), (sb_wx, d_wx), (sb_iden, d_iden),
                          (sb_f1w, d_f1w), (sb_f1b, d_f1b), (sb_f2w, d_f2w),
                          (sb_f2b, d_f2b)):
            nc.sync.dma_start(dst[:], dsrc.ap())

        sx_tile = None

        def emit_x(tau):
            """x-part + bias for step tau: one bank-wide MM per bank.

            start=True clears has_written for the WHOLE bank, so the clear
            must come from a single MM covering all 128 partitions. The K=6
            stationary is [x;1;1;0;0;0] for rows 0:64 and [0;0;0;x;1;1] for
            rows 64:128; moving rows 0-2 / 3-5 carry the two col-halves'
            [w_ih; b_hi; b_lo]."""
            nonlocal sx_tile, banks
            if tau % TCH == 0:
                sx_tile = seqp.tile([6, TCH * 128], BF16, tag="sx")
                nc.sync.dma_start(sx_tile[:],
                                  d_sx.ap()[:, tau * 128:(tau + TCH) * 128])
            tt = tau % TCH
            bIF = pg.tile([128, 512], F32, tag="IF")
            bGO = pg.tile([128, 512], F32, tag="GO")
            banks = (bIF, bGO)
            stop = (tau == 0)   # step 0 has no h accumulation
            for bk in range(2):
                nc.tensor.matmul(
                    banks[bk][:],
                    sx_tile[:][:, tt * 128:(tt + 1) * 128],
                    sb_wx[:][:, bk * 512:(bk + 1) * 512],
                    start=True, stop=stop)

        banks = None
        emit_x(0)

        c_t = None
        hT_L = hT_R = None

        for t in range(T):
            bIF, bGO = banks
            if t > 0:
                # recurrent matmuls: gate-major, L K-chunks first; col-paired
                for g in GATES:
                    bk, ch = GBANK[g]
                    gt = banks[bk][:][:, ch * 256:(ch + 1) * 256]
                    for k in KORDER:
                        hT = hT_L if k in (0, 2) else hT_R
                        co = 0 if k in (0, 1) else 128
                        woff = ((GATES.index(g) * 4 + k) * 2) * 256
                        for hf in range(2):
                            nc.tensor.matmul(
                                gt[hf * 64:(hf + 1) * 64, :],
                                hT[:][:, co + hf * 64: co + (hf + 1) * 64],
                                sb_whh[:][:, woff + hf * 256: woff + (hf + 1) * 256],
                                start=False, stop=(k == 3))

            # ---- elementwise chain (ACT + DVE), [128, 128] halves
            sI = acts.tile([128, 256], BF16, tag="sI")
            tG = acts.tile([128, 256], BF16, tag="tG")
            sF = acts.tile([128, 256], F32, tag="sF")
            sO = acts.tile([128, 256], BF16, tag="sO")
            Tc = acts.tile([128, 256], BF16, tag="Tc")
            t1 = acts.tile([128, 256], F32, tag="t1")
            t2 = acts.tile([128, 256], F32, tag="t2")
            c = state.tile([128, 256], F32, tag="c")
            h = state.tile([128, 256], BF16, tag="h")

            gI = bIF[:][:, 0:256]
            gF = bIF[:][:, 256:512]
            gG = bGO[:][:, 0:256]
            gO = bGO[:][:, 256:512]

            nc.scalar.activation(sI[:], gI, AF.Sigmoid)
            nc.scalar.activation(tG[:][:, 0:128], gG[:, 0:128], AF.Tanh)
            nc.scalar.activation(tG[:][:, 128:256], gG[:, 128:256], AF.Tanh)
            if t > 0:
                nc.scalar.activation(sF[:][:, 0:128], gF[:, 0:128], AF.Sigmoid)
                nc.scalar.activation(sF[:][:, 128:256], gF[:, 128:256], AF.Sigmoid)
            nc.scalar.activation(sO[:], gO, AF.Sigmoid)

            ctgt = c if t == 0 else t1
            for hx in range(2):
                sl = slice(hx * 128, (hx + 1) * 128)
                nc.vector.tensor_mul(ctgt[:][:, sl], sI[:][:, sl], tG[:][:, sl])
            if t > 0:
                for hx in range(2):
                    sl = slice(hx * 128, (hx + 1) * 128)
                    nc.vector.tensor_mul(t2[:][:, sl], sF[:][:, sl], c_t[:][:, sl])
                    nc.vector.tensor_add(c[:][:, sl], t1[:][:, sl], t2[:][:, sl])

            for hx in range(2):
                sl = slice(hx * 128, (hx + 1) * 128)
                nc.scalar.activation(Tc[:][:, sl], c[:][:, sl], AF.Tanh)
                nc.vector.tensor_mul(h[:][:, sl], sO[:][:, sl], Tc[:][:, sl])

            # x-part for t+1 fills PE while the chain runs
            if t + 1 < T:
                emit_x(t + 1)

            # transposes: h[0:64, f] = cols f (chunks 0/1), h[64:128, f] = cols
            # 256+f (chunks 2/3); [I64|I64] identity -> duplicated stationary.
            # Base-0 and base-64 stationaries must NOT share a PSUM tile
            # (mixed row-group matmuls into one tile crash the runtime).
            pT0 = pg.tile([128, 256], BF16, tag="pT0", bufs=1)   # chunks 0, 1
            pT1 = pg.tile([128, 256], BF16, tag="pT1", bufs=1)   # chunks 2, 3
            nc.tensor.transpose(pT0[:][:, 0:128], h[:][0:64, 0:128],
                                sb_iden[:][0:64, :])
            nc.tensor.transpose(pT1[:][:, 0:128], h[:][64:128, 0:128],
                                sb_iden[:][64:128, :])
            nc.tensor.transpose(pT0[:][:, 128:256], h[:][0:64, 128:256],
                                sb_iden[:][0:64, :])
            nc.tensor.transpose(pT1[:][:, 128:256], h[:][64:128, 128:256],
                                sb_iden[:][64:128, :])

            hT_L = state.tile([128, 256], BF16, tag="hTL")
            hT_R = state.tile([128, 256], BF16, tag="hTR")
            # hT_L = chunks {0, 2}, hT_R = chunks {1, 3}
            nc.vector.tensor_copy(hT_L[:][:, 0:128], pT0[:][:, 0:128])
            nc.vector.tensor_copy(hT_L[:][:, 128:256], pT1[:][:, 0:128])
            nc.vector.tensor_copy(hT_R[:][:, 0:128], pT0[:][:, 128:256])
            nc.vector.tensor_copy(hT_R[:][:, 128:256], pT1[:][:, 128:256])

            if DBG is not None and t == DBG:
                hf32 = acts.tile([128, 256], F32, tag="dbgh", bufs=1)
                nc.vector.tensor_copy(hf32[:], h[:])
                nc.sync.dma_start(d_dbg_h.ap(), hf32[:])
                nc.sync.dma_start(d_dbg_c.ap(), c[:])
                gf32 = acts.tile([128, 1024], F32, tag="dbgg", bufs=1)
                nc.vector.tensor_copy(gf32[:][:, 0:512], bIF[:])
                nc.vector.tensor_copy(gf32[:][:, 512:1024], bGO[:])
                nc.sync.dma_start(d_dbg_g.ap(), gf32[:])

            c_t = c

        # ---- FC head: hid = relu(h @ fc1_w.T + b1); out = hid @ fc2_w.T + b2
        ones_f = acts.tile([2, 64], F32, tag="onesf", bufs=2)
        ones_b = acts.tile([2, 64], BF16, tag="onesb", bufs=2)
        nc.gpsimd.memset(ones_f[:], 1.0)
        nc.vector.tensor_copy(ones_b[:], ones_f[:])
        ones = ones_b[:]
        p_hid = pg.tile([64, HALF], F32, tag="IF")
        nc.tensor.matmul(p_hid[:], ones, sb_f1b[:], start=True, stop=False)
        # stationary: single (non-dup) hT chunk slices; chunks 0,2 in hT_L
        for k in range(4):
            hT = hT_L if k in (0, 2) else hT_R
            co = 0 if k in (0, 1) else 128
            nc.tensor.matmul(p_hid[:], hT[:][:, co:co + 64],
                             sb_f1w[:][:, k * HALF:(k + 1) * HALF],
                             start=False, stop=(k == 3))
        hid = acts.tile([64, HALF], BF16, tag="hid")
        nc.scalar.activation(hid[:], p_hid[:], AF.Relu)

        pTh = pg.tile([128, 128], BF16, tag="GO")
        nc.tensor.transpose(pTh[:][:, 0:64], hid[:][:, 0:128],
                            sb_iden[:][0:64, 0:64])
        nc.tensor.transpose(pTh[:][:, 64:128], hid[:][:, 128:256],
                            sb_iden[:][0:64, 0:64])
        hidT = acts.tile([128, 128], BF16, tag="hidT")
        nc.vector.tensor_copy(hidT[:], pTh[:])

        p_out = pg.tile([64, TGT], F32, tag="pTT", bufs=1)
        nc.tensor.matmul(p_out[:], ones, sb_f2b[:], start=True, stop=False)
        for k in range(2):
            nc.tensor.matmul(p_out[:], hidT[:][:, k * 64:(k + 1) * 64],
                             sb_f2w[:][:, k * TGT:(k + 1) * TGT],
                             start=False, stop=(k == 1))
        res = acts.tile([BL, TGT], F32, tag="res")
        nc.vector.tensor_copy(res[:], p_out[:])
        nc.sync.dma_start(d_out.ap(), res[:])

    if not nc.is_finalized():
        nc.finalize()
    return nc


def _bf(x):
    return np.asarray(x, np.float32).astype(BFNP)


def _prep_shared(W_ih, W_hh, b_ih, b_hh, fc1_w, fc1_b, fc2_w, fc2_b):
    W_hh = np.asarray(W_hh, np.float32)
    wih = np.asarray(W_ih, np.float32)[:, 0]
    bias = np.asarray(b_ih, np.float32) + np.asarray(b_hh, np.float32)
    b_hi = bias.astype(BFNP).astype(np.float32)
    b_lo = bias - b_hi

    whh = np.empty((128, 8192), BFNP)
    for gi, g in enumerate(GATES):
        for k in range(4):
            for hf in range(2):
                off = ((gi * 4 + k) * 2 + hf) * 256
                rows = GROW[g] + hf * 256
                whh[:, off:off + 256] = _bf(
                    W_hh[rows:rows + 256, k * 128:(k + 1) * 128].T)

    wx = np.empty((6, 1024), BFNP)
    for bk, (g1, g2) in enumerate((("i", "f"), ("g", "o"))):
        for hf in range(2):            # hf -> moving row group (out row half)
            for gj, g in enumerate((g1, g2)):
                off = bk * 512 + gj * 256
                rows = slice(GROW[g] + hf * 256, GROW[g] + hf * 256 + 256)
                wx[hf * 3 + 0, off:off + 256] = _bf(wih[rows])
                wx[hf * 3 + 1, off:off + 256] = _bf(b_hi[rows])
                wx[hf * 3 + 2, off:off + 256] = _bf(b_lo[rows])

    iden = np.zeros((128, 128), np.float32)
    ii = np.arange(128)
    iden[ii[:, None] % 64 == ii[None, :] % 64] = 1.0

    f1w = np.empty((128, 1024), BFNP)
    for k in range(4):
        f1w[:, k * HALF:(k + 1) * HALF] = _bf(
            np.asarray(fc1_w, np.float32)[:, k * 128:(k + 1) * 128].T)
    b1 = np.asarray(fc1_b, np.float32)
    b1_hi = b1.astype(BFNP).astype(np.float32)
    f1b = np.stack([b1_hi, b1 - b1_hi]).astype(BFNP)

    f2w = np.empty((128, 2 * TGT), BFNP)
    for k in range(2):
        f2w[:, k * TGT:(k + 1) * TGT] = _bf(
            np.asarray(fc2_w, np.float32)[:, k * 128:(k + 1) * 128].T)
    b2 = np.asarray(fc2_b, np.float32)
    b2_hi = b2.astype(BFNP).astype(np.float32)
    f2b = np.stack([b2_hi, b2 - b2_hi]).astype(BFNP)

    return {"whh": whh, "wx": wx, "iden": iden.astype(BFNP),
            "f1w": f1w, "f1b": f1b, "f2w": f2w, "f2b": f2b}


def run(inputs, trace=False):
    if "nc" not in _cached:
        _cached["nc"] = build_program()
    nc = _cached["nc"]
    shared = _prep_shared(
        inputs["W_ih"], inputs["W_hh"], inputs["b_ih"], inputs["b_hh"],
        inputs["fc1_w"], inputs["fc1_b"], inputs["fc2_w"], inputs["fc2_b"])
    seq = np.asarray(inputs["sequence"], np.float32)[:, :, 0]   # [B, T]
    in_maps = []
    for cid in range(NCORES):
        xs = seq[cid * BL:(cid + 1) * BL, :].T                  # [T, 64]
        z = np.zeros_like(xs)
        o = np.ones_like(xs)
        # per step block of 128 cols: rows 0-2 = [x;1;1 | 0], rows 3-5 = [0 | x;1;1]
        sx = np.stack([
            np.concatenate([xs, z], 1), np.concatenate([o, z], 1),
            np.concatenate([o, z], 1), np.concatenate([z, xs], 1),
            np.concatenate([z, o], 1), np.concatenate([z, o], 1),
        ]).reshape(6, T * 128).astype(BFNP)
        in_maps.append({"sx": sx, **shared})
    br = run_bass_kernel_spmd(nc, in_maps, list(range(NCORES)), trace=trace)
    out = np.concatenate([br.results[i]["out"] for i in range(NCORES)], axis=0)
    return out[:, :, None].astype(np.float32), br


def kernel(**inputs):
    out, _ = run(inputs)
    return out
